# revision 26
# baseline (speedup 1.0000x reference)
"""Canny edge detection on 8 Trainium2 NeuronCores (Bass/Tile).

Self-contained: shards the full 2048x2048 input across 8 cores (row blocks
with halos), runs one SPMD Bass kernel, gathers the full (3,2048,2048) output.

v2: full-width NMS with engine-balanced ops, DMA column shifts, packed u32
output unpacked on host.
"""
import numpy as np
from contextlib import ExitStack

import concourse.bass as bass
import concourse.bacc as bacc
import concourse.tile as tile
import concourse.mybir as mybir
from concourse.alu_op_type import AluOpType as Op
from concourse.bass_utils import run_bass_kernel_spmd

F32 = mybir.dt.float32
F16 = mybir.dt.float16
I32 = mybir.dt.int32
U32 = mybir.dt.uint32
U16 = mybir.dt.uint16
AF = mybir.ActivationFunctionType

H_IMG, W_IMG = 2048, 2048
N_CORES = 8
OUT_ROWS = H_IMG // N_CORES          # 256
T_ITERS = 8                           # fixed masked-dilate iterations
R_IMG = 276                           # local img rows
Y0, Y1 = 2, 274                       # img rows with weak/strong (272 rows)
RW = Y1 - Y0                          # 272
HH = RW + 2                           # 274: hysteresis rows (1 zero pad each side)
BASE_OFF = 10                         # local img row of first output row
NCHUNK = W_IMG // 128                 # 16 column chunks
NSTRIP = W_IMG // 16                  # 128 strips of 16 cols (+8 halo each side)
T1 = float(np.sqrt(2.0) - 1.0)        # tan(22.5 deg)
W_PAD = W_IMG + 2                     # 2050 (1 replicated col each side)
M23 = float(2 ** 23)
CS = [128, NCHUNK, R_IMG]             # col-layout shape


# ---------------------------------------------------------------- host consts
def _make_consts():
    c = {}
    # Vertical band matrices: out[n] = sum_k B[k, n] * in[global_row(k)]
    # B121: [1,2,1] smoothing; B101: out[n] = in[n+1] - in[n-1]
    b121 = np.zeros((128, 3, R_IMG), np.float16)
    b101 = np.zeros((128, 3, R_IMG), np.float16)
    for rc in range(3):
        for k in range(128):
            gr = 128 * rc + k
            if gr >= R_IMG:
                continue
            for n in range(1, R_IMG - 1):
                d = gr - n
                if d == -1 or d == 1:
                    b121[k, rc, n] = 1.0
                elif d == 0:
                    b121[k, rc, n] = 2.0
                if d == 1:
                    b101[k, rc, n] = 1.0
                elif d == -1:
                    b101[k, rc, n] = -1.0
    c["b121"] = b121
    c["b101"] = b101

    # Bit-pack matrices: strip s covers cols 16s-8 .. 16s+23 (bit b = col 16s-8+b)
    wlo = np.zeros((128, NCHUNK, 128), np.float16)
    whi = np.zeros((128, NCHUNK, 128), np.float16)
    for j in range(NCHUNK):
        for k in range(128):
            col = 128 * j + k
            for s in range(NSTRIP):
                b = col - 16 * s + 8
                if 0 <= b < 16:
                    wlo[k, j, s] = float(2 ** b)
                elif 16 <= b < 32:
                    whi[k, j, s] = float(2 ** (b - 16))
    c["wlo"] = wlo
    c["whi"] = whi
    return c


_CONSTS = None


def _consts():
    global _CONSTS
    if _CONSTS is None:
        _CONSTS = _make_consts()
    return _CONSTS


def _host_shards(x):
    """Per-core input shards + per-row uint32 penalty mask [128, HH]."""
    x = np.asarray(x, dtype=np.float32)
    shards = []
    for c in range(N_CORES):
        base = OUT_ROWS * c - BASE_OFF
        rows = np.clip(np.arange(base, base + R_IMG), 0, H_IMG - 1)
        xs = np.pad(x[rows], ((0, 0), (1, 1)), mode="edge").astype(np.float32)
        # hysteresis row h (1..HH-2) <-> img-local row h+1, global base+h+1
        glob = base + np.arange(HH) + 1
        ok = (glob >= 1) & (glob <= H_IMG - 2)
        ok[0] = False
        ok[HH - 1] = False
        pen = np.where(ok, np.uint32(0xFFFFFFFF), np.uint32(0))
        penrep = np.broadcast_to(pen[None, :], (128, HH)).copy()
        penrep[0, :] &= np.uint32(~(1 << 8) & 0xFFFFFFFF)     # col 0 border
        penrep[127, :] &= np.uint32(~(1 << 23) & 0xFFFFFFFF)  # col 2047 border
        shards.append((xs, penrep))
    return shards


# ---------------------------------------------------------------- device body
def _body(tc: tile.TileContext, io):
    nc = tc.nc
    x_d, pen_d, b121_d, b101_d, wlo_d, whi_d, out_d = io[:7]
    dbg = io[7] if len(io) > 7 else None
    R = R_IMG
    NQ = 4
    GQ = NCHUNK // NQ

    rc_rows = [(0, 128), (128, 128), (256, R - 256)]

    with ExitStack() as outer:
        singles = outer.enter_context(tc.tile_pool(name="consts", bufs=1))
        pfull = outer.enter_context(tc.tile_pool(name="pfull", bufs=1))
        pA = outer.enter_context(tc.tile_pool(name="pA", bufs=2))
        pAC = outer.enter_context(tc.tile_pool(name="pAC", bufs=3))
        pC = outer.enter_context(tc.tile_pool(name="pC", bufs=2))
        ph = outer.enter_context(tc.tile_pool(name="ph", bufs=1))
        pSh = outer.enter_context(tc.tile_pool(name="pSh", bufs=2))
        phorS = ExitStack()
        phor = phorS.enter_context(tc.tile_pool(name="phor", bufs=1))
        psumS = ExitStack()
        psum1 = psumS.enter_context(tc.tile_pool(name="psum1", bufs=2,
                                                 space="PSUM"))
        ppckS = ExitStack()
        ppck = ppckS.enter_context(tc.tile_pool(name="psumpk", bufs=1,
                                                space="PSUM"))

        # full-width persistent tiles
        mag = pfull.tile(CS, F16, tag="mag")

        # ------- phase 1: load, integerize (exact floor(255x)), horiz passes
        dT = phor.tile([128, 3, W_IMG], F16, tag="dT")
        sT = phor.tile([128, 3, W_IMG], F16, tag="sT")
        with ExitStack() as ph1:
            px = ph1.enter_context(tc.tile_pool(name="px", bufs=2))
            pw = ph1.enter_context(tc.tile_pool(name="pw", bufs=3))
            pimg = ph1.enter_context(tc.tile_pool(name="pimg", bufs=1))

            imgf = None
            if dbg is not None:
                imgf = pimg.tile([128, 3, W_PAD], F16, name="imgf",
                                 tag="img")
            for rc in (2, 0, 1):
                r0, nr = rc_rows[rc]
                img = None
                if dbg is None:
                    img = pimg.tile([128, W_PAD], F16, name="imgc",
                                    tag="imgc", bufs=2)
                if dbg is not None:
                    def imgv(a, b, rc=rc):
                        return imgf[:a, rc, b]
                else:
                    def imgv(a, b, img=img):
                        return img[:a, b]
                # column halves: integerize [0:1026] then [1026:2050]
                for cs, ce in ((0, 1026), (1026, W_PAD)):
                    ln = ce - cs
                    xt = px.tile([128, 1026], F32, tag="x")
                    nc.sync.dma_start(xt[:nr, :ln], x_d[r0:r0 + nr, cs:ce])
                    # y = fl(255*x) exactly as the reference computes it
                    yt = pw.tile([128, 1026], F32, tag="y")
                    nc.scalar.activation(yt[:nr, :ln], xt[:nr, :ln], AF.Copy,
                                         scale=255.0)
                    # exact floor(y): n = rne(y), img = n - (n > y)
                    n16 = pw.tile([128, 1026], F16, tag="n16")
                    if rc != 1:
                        nA = pw.tile([128, 1026], F32, tag="nA")
                        nc.scalar.activation(nA[:nr, :ln], yt[:nr, :ln],
                                             AF.Copy, bias=M23)
                        nc.scalar.activation(n16[:nr, :ln], nA[:nr, :ln],
                                             AF.Copy, bias=-M23)
                    else:
                        nc.vector.tensor_scalar(n16[:nr, :ln], yt[:nr, :ln],
                                                M23, M23, Op.add, Op.subtract)
                    d16 = pw.tile([128, 1026], U16, tag="d16")
                    nc.vector.tensor_tensor(d16[:nr, :ln], n16[:nr, :ln],
                                            yt[:nr, :ln], Op.is_gt)
                    nc.vector.tensor_tensor(imgv(nr, slice(cs, ce)),
                                            n16[:nr, :ln], d16[:nr, :ln],
                                            Op.subtract)
                    # horizontal passes for the covered window
                    ws, we = (0, 1024) if cs == 0 else (1024, W_IMG)
                    wl = we - ws
                    nc.vector.tensor_tensor(dT[:nr, rc, ws:we],
                                            imgv(nr, slice(ws + 2, we + 2)),
                                            imgv(nr, slice(ws, we)),
                                            Op.subtract)
                    c2 = pw.tile([128, 1024], F16, tag="c2")
                    nc.scalar.activation(c2[:nr, :wl],
                                         imgv(nr, slice(ws + 1, we + 1)),
                                         AF.Copy, scale=2.0)
                    s1t = pw.tile([128, 1024], F16, tag="s1")
                    nc.gpsimd.tensor_tensor(s1t[:nr, :wl],
                                            imgv(nr, slice(ws, we)),
                                            imgv(nr, slice(ws + 2, we + 2)),
                                            Op.add)
                    nc.vector.tensor_tensor(sT[:nr, rc, ws:we],
                                            s1t[:nr, :wl], c2[:nr, :wl],
                                            Op.add)

            # ---- constants to SBUF (queued behind x loads on purpose)
            b121 = singles.tile([128, 3, R], F16)
            nc.sync.dma_start(b121[:], b121_d)
            b101 = singles.tile([128, 3, R], F16)
            nc.sync.dma_start(b101[:], b101_d)
            wlo = singles.tile([128, NCHUNK, 128], F16)
            nc.sync.dma_start(wlo[:], wlo_d)
            whi = singles.tile([128, NCHUNK, 128], F16)
            nc.sync.dma_start(whi[:], whi_d)
            pen = singles.tile([128, HH], U32)
            nc.sync.dma_start(pen[:], pen_d)
            sc1 = singles.tile([128, 1], U32)
            nc.vector.memset(sc1[:], 1)
            sc16 = singles.tile([128, 1], U32)
            nc.vector.memset(sc16[:], 16)
            zrow = singles.tile([1, R], F16)
            nc.vector.memset(zrow[:], 0.0)
            if dbg is not None:
                nc.sync.dma_start(dbg["img"], imgf[:])

        # pack accumulators (PSUM, accumulate across all 16 chunks)
        pk_wklo = ppck.tile([128, RW], F32, tag="wklo")
        pk_wkhi = ppck.tile([128, RW], F32, tag="wkhi")
        pk_stlo = ppck.tile([128, RW], F32, tag="stlo")
        pk_sthi = ppck.tile([128, RW], F32, tag="sthi")

        # ------- per-group NMS pipeline
        # /64-scaled domain: all magnitudes are k/64 (exact in f16, k<=2040)
        grp = [None] * NQ   # rotating per-group tiles

        def stage_a(q):
            j0 = GQ * q
            gxs = pA.tile([128, GQ, R], F16, tag="gxs")
            gys = pA.tile([128, GQ, R], F16, tag="gys")
            for jj in range(GQ):
                j = j0 + jj
                gxp = psum1.tile([128, R], F32, tag="gx")
                gyp = psum1.tile([128, R], F32, tag="gy")
                for rc, (r0, nr) in enumerate(rc_rows):
                    nc.tensor.matmul(gxp[:],
                                     dT[:nr, rc, 128 * j:128 * (j + 1)],
                                     b121[:nr, rc, :], start=(rc == 0),
                                     stop=(rc == 2))
                for rc, (r0, nr) in enumerate(rc_rows):
                    nc.tensor.matmul(gyp[:],
                                     sT[:nr, rc, 128 * j:128 * (j + 1)],
                                     b101[:nr, rc, :], start=(rc == 0),
                                     stop=(rc == 2))
                nc.scalar.activation(gxs[:, jj, :], gxp[:], AF.Copy,
                                     scale=1.0 / 64.0)
                nc.scalar.activation(gys[:, jj, :], gyp[:], AF.Copy,
                                     scale=1.0 / 64.0)
            absx = pA.tile([128, GQ, R], F16, tag="absx")
            nc.scalar.activation(absx[:], gxs[:], AF.Abs)
            absy = pA.tile([128, GQ, R], F16, tag="absy")
            nc.scalar.activation(absy[:], gys[:], AF.Abs)
            pm = pA.tile([128, GQ, R], F16, tag="pm")
            nc.gpsimd.tensor_tensor(pm[:], gxs[:], gys[:], Op.mult)
            sl = slice(j0, j0 + GQ)
            nc.vector.tensor_tensor(mag[:, sl, :], absx[:], absy[:], Op.add)

            d0 = pAC.tile([128, GQ, R], U16, tag="d0")
            nc.vector.scalar_tensor_tensor(d0[:], absx[:], T1, absy[:],
                                           Op.mult, Op.is_gt)
            d2 = pAC.tile([128, GQ, R], U16, tag="d2")
            nc.vector.scalar_tensor_tensor(d2[:], absy[:], T1, absx[:],
                                           Op.mult, Op.is_ge)
            pmneg = pAC.tile([128, GQ, R], U16, tag="pmneg")
            nc.vector.tensor_single_scalar(pmneg[:], pm[:], 0.0, Op.is_lt)
            grp[q] = {"d0": d0, "d2": d2, "pmneg": pmneg}

        def shifts(q):
            j0, j1 = GQ * q, GQ * (q + 1)
            for sa, tl, tr in ((mag, "magL", "magR"),):
                dl = pSh.tile([128, GQ, R], F16, name=tl + str(q), tag=tl)
                dr = pSh.tile([128, GQ, R], F16, name=tr + str(q), tag=tr)
                grp[q][tl] = dl
                grp[q][tr] = dr
                nc.sync.dma_start(dr[0:127, :, :], sa[1:128, j0:j1, :])
                nc.sync.dma_start(dr[127:128, 0:GQ - 1, :],
                                  sa[0:1, j0 + 1:j1, :])
                if q < NQ - 1:
                    nc.sync.dma_start(dr[127:128, GQ - 1:GQ, :],
                                      sa[0:1, j1:j1 + 1, :])
                else:
                    nc.sync.dma_start(dr[127:128, GQ - 1:GQ, :], zrow[:])
                nc.sync.dma_start(dl[1:128, :, :], sa[0:127, j0:j1, :])
                nc.sync.dma_start(dl[0:1, 1:GQ, :],
                                  sa[127:128, j0:j1 - 1, :])
                if q > 0:
                    nc.sync.dma_start(dl[0:1, 0:1, :],
                                      sa[127:128, j0 - 1:j0, :])
                else:
                    nc.vector.memset(dl[0:1, 0:1, :], 0.0)

        def G(t, q, dy=0):
            return t[:, GQ * q:GQ * (q + 1), Y0 + dy:Y1 + dy]

        def stage_c(q):
            g = grp[q]
            Mt = pC.tile([128, GQ, R], F16, tag="Mt")
            Mi3 = pC.tile([128, GQ, R], F16, tag="Mi3")
            M0 = pC.tile([128, GQ, R], F16, tag="M0")
            M2 = pC.tile([128, GQ, R], F16, tag="M2")

            def YG(t, dy=0):
                return t[:, :, Y0 + dy:Y1 + dy]

            mL, mR = g["magL"], g["magR"]
            qL = pC.tile([128, GQ, R], F16, tag="qL")
            nc.vector.tensor_scalar(qL[:], mL[:], 1.0 / 64.0, None, Op.add)
            qR = pC.tile([128, GQ, R], F16, tag="qR")
            nc.vector.tensor_scalar(qR[:], mR[:], 1.0 / 64.0, None, Op.add)
            qC = pC.tile([128, GQ, R], F16, tag="qC")
            nc.vector.tensor_scalar(qC[:], mag[:, GQ * q:GQ * (q + 1), :],
                                    1.0 / 64.0, None, Op.add)
            nc.vector.tensor_tensor(YG(Mt), YG(qL, 1), YG(mR, -1),
                                    Op.max)                              # 45
            nc.vector.tensor_tensor(YG(Mi3), YG(qR, 1), YG(mL, -1),
                                    Op.max)                              # 135
            nc.vector.tensor_tensor(YG(M0), YG(qL), YG(mR), Op.max)
            nc.vector.tensor_tensor(YG(M2), YG(qC, 1), G(mag, q, -1),
                                    Op.max)                              # 90
            nc.vector.copy_predicated(YG(Mt), YG(g["pmneg"]), YG(Mi3))
            nc.vector.copy_predicated(YG(Mt), YG(g["d0"]), YG(M0))
            nc.vector.copy_predicated(YG(Mt), YG(g["d2"]), YG(M2))
            Mw = pC.tile([128, GQ, R], F16, tag="Mw")
            nc.vector.tensor_scalar(YG(Mw), YG(Mt), 101.0 / 64.0, None,
                                    Op.max)
            weak = pC.tile([128, GQ, R], F16, tag="M0")
            nc.vector.tensor_tensor(YG(weak), YG(Mw), G(mag, q), Op.is_le)
            nc.vector.tensor_scalar(YG(Mi3), YG(Mw), 201.0 / 64.0, None,
                                    Op.max)
            strong = pC.tile([128, GQ, R], F16, tag="M2")
            nc.vector.tensor_tensor(YG(strong), YG(Mi3), G(mag, q), Op.is_le)

            j0 = GQ * q
            for jj in range(GQ):
                j = j0 + jj
                st_j = (j == 0)
                sp_j = (j == NCHUNK - 1)
                nc.tensor.matmul(pk_wklo[:], wlo[:, j, :],
                                 weak[:, jj, Y0:Y1], start=st_j, stop=sp_j,
                                 skip_group_check=True)
                nc.tensor.matmul(pk_wkhi[:], whi[:, j, :],
                                 weak[:, jj, Y0:Y1], start=st_j, stop=sp_j,
                                 skip_group_check=True)
                nc.tensor.matmul(pk_stlo[:], wlo[:, j, :],
                                 strong[:, jj, Y0:Y1], start=st_j, stop=sp_j,
                                 skip_group_check=True)
                nc.tensor.matmul(pk_sthi[:], whi[:, j, :],
                                 strong[:, jj, Y0:Y1], start=st_j, stop=sp_j,
                                 skip_group_check=True)

        # staggered emission: stageA(q+1) before shifts(q) (cross-group edges)
        stage_a(0)
        stage_a(1)
        shifts(0)
        stage_a(2)
        shifts(1)
        stage_c(0)
        stage_a(3)
        shifts(2)
        stage_c(1)
        shifts(3)
        stage_c(2)
        stage_c(3)
        phorS.close()

        # ------- combine packed halves into u32 words, apply border penalty
        wk32 = ph.tile([128, HH], U32, tag="wk")
        st32 = ph.tile([128, HH], U32, tag="st")
        lo32 = ph.tile([128, RW], U32, tag="lo32")
        hi32 = ph.tile([128, RW], U32, tag="hi32")
        nc.vector.tensor_copy(lo32[:], pk_wklo[:])
        nc.vector.tensor_copy(hi32[:], pk_wkhi[:])
        nc.vector.scalar_tensor_tensor(wk32[:, 1:HH - 1], hi32[:], sc16[:],
                                       lo32[:], Op.logical_shift_left,
                                       Op.bitwise_or)
        lo32b = ph.tile([128, RW], U32, tag="lo32b")
        hi32b = ph.tile([128, RW], U32, tag="hi32b")
        nc.vector.tensor_copy(lo32b[:], pk_stlo[:])
        nc.vector.tensor_copy(hi32b[:], pk_sthi[:])
        nc.vector.scalar_tensor_tensor(st32[:, 1:HH - 1], hi32b[:], sc16[:],
                                       lo32b[:], Op.logical_shift_left,
                                       Op.bitwise_or)
        ppckS.close()
        psumS.close()
        nc.vector.tensor_tensor(wk32[:, 1:HH - 1], wk32[:, 1:HH - 1],
                                pen[:, 1:HH - 1], Op.bitwise_and)
        nc.vector.tensor_tensor(st32[:, 1:HH - 1], st32[:, 1:HH - 1],
                                pen[:, 1:HH - 1], Op.bitwise_and)

        # ------- hysteresis: fixed masked-dilate iterations on packed words
        cur = st32
        nxt = ph.tile([128, HH], U32, tag="curB")
        at = ph.tile([128, HH], U32, tag="a")
        bt = ph.tile([128, HH], U32, tag="b")
        ut = ph.tile([128, HH], U32, tag="u")
        nc.vector.memset(bt[:, 0:1], 0)
        nc.vector.memset(bt[:, HH - 1:HH], 0)
        for it in range(T_ITERS):
            nc.vector.scalar_tensor_tensor(
                at[:, 1:HH - 1], cur[:, 1:HH - 1], sc1[:], cur[:, 1:HH - 1],
                Op.logical_shift_left, Op.bitwise_or)
            nc.vector.scalar_tensor_tensor(
                bt[:, 1:HH - 1], cur[:, 1:HH - 1], sc1[:], at[:, 1:HH - 1],
                Op.logical_shift_right, Op.bitwise_or)
            nc.vector.tensor_tensor(ut[:, 1:HH - 1], bt[:, 0:HH - 2],
                                    bt[:, 2:HH], Op.bitwise_or)
            nc.vector.tensor_tensor(ut[:, 1:HH - 1], ut[:, 1:HH - 1],
                                    bt[:, 1:HH - 1], Op.bitwise_or)
            nc.vector.tensor_tensor(nxt[:, 1:HH - 1], ut[:, 1:HH - 1],
                                    wk32[:, 1:HH - 1], Op.bitwise_and)
            cur, nxt = nxt, cur

        if dbg is not None:
            nc.sync.dma_start(dbg["wk32"], wk32[:])
            nc.sync.dma_start(dbg["st32"], st32[:])
            nc.sync.dma_start(dbg["mag"], mag[:])

        # ------- emit packed output rows (host unpacks bits)
        nc.sync.dma_start(out_d, cur[:, BASE_OFF - 1:BASE_OFF - 1 + OUT_ROWS])


def _build_nc(debug_out=False):
    nc = bacc.Bacc("TRN2", target_bir_lowering=False, debug=False,
                   num_devices=N_CORES)
    x_d = nc.dram_tensor("x", [R_IMG, W_PAD], F32, kind="ExternalInput").ap()
    pen_d = nc.dram_tensor("pen", [128, HH], U32, kind="ExternalInput").ap()
    b121_d = nc.dram_tensor("b121", [128, 3, R_IMG], F16, kind="ExternalInput").ap()
    b101_d = nc.dram_tensor("b101", [128, 3, R_IMG], F16, kind="ExternalInput").ap()
    wlo_d = nc.dram_tensor("wlo", [128, NCHUNK, 128], F16, kind="ExternalInput").ap()
    whi_d = nc.dram_tensor("whi", [128, NCHUNK, 128], F16, kind="ExternalInput").ap()
    out_d = nc.dram_tensor("out", [128, OUT_ROWS], U32, kind="ExternalOutput").ap()
    io = [x_d, pen_d, b121_d, b101_d, wlo_d, whi_d, out_d]
    if debug_out:
        dbg = {}
        for nm, shp, dt in [("wk32", [128, HH], U32), ("st32", [128, HH], U32),
                            ("mag", CS, F16),
                            ("img", [128, 3, W_PAD], F16)]:
            dbg[nm] = nc.dram_tensor("dbg_" + nm, shp, dt,
                                     kind="ExternalOutput").ap()
        io.append(dbg)
    with tile.TileContext(nc) as tc:
        _body(tc, io)
    nc.compile()
    return nc


_NC = None


def _get_nc():
    global _NC
    if _NC is None:
        _NC = _build_nc()
    return _NC


def _in_maps(x):
    cs = _consts()
    shards = _host_shards(x)
    maps = []
    for c in range(N_CORES):
        xs, pen = shards[c]
        maps.append({
            "x": xs, "pen": pen,
            "b121": cs["b121"], "b101": cs["b101"],
            "wlo": cs["wlo"], "whi": cs["whi"],
        })
    return maps


def _unpack_words(words):
    """[128, 256] u32 strip words -> [256, 2048] f32 edge map."""
    w = np.ascontiguousarray(words.astype('<u4'))
    by = w.view(np.uint8).reshape(128, OUT_ROWS, 4)[:, :, 1:3]
    bits = np.unpackbits(by, axis=2, bitorder="little")  # [128, 256, 16]
    return np.transpose(bits, (1, 0, 2)).reshape(OUT_ROWS, W_IMG)


LAST_RESULT = None


def kernel(x):
    global LAST_RESULT
    nc = _get_nc()
    maps = _in_maps(x)
    res = run_bass_kernel_spmd(nc, maps, list(range(N_CORES)))
    LAST_RESULT = res
    blocks = [_unpack_words(res.results[c]["out"]) for c in range(N_CORES)]
    edges = np.concatenate(blocks, axis=0).astype(np.float32)
    return np.broadcast_to(edges[None], (3, H_IMG, W_IMG))


# revision 27
# speedup vs baseline: 1.0661x; 1.0661x over previous
"""Canny edge detection on 8 Trainium2 NeuronCores (Bass/Tile).

Self-contained: shards the full 2048x2048 input across 8 cores (row blocks
with halos), runs one SPMD Bass kernel, gathers the full (3,2048,2048) output.

v2: full-width NMS with engine-balanced ops, DMA column shifts, packed u32
output unpacked on host.
"""
import numpy as np
from contextlib import ExitStack

import concourse.bass as bass
import concourse.bacc as bacc
import concourse.tile as tile
import concourse.mybir as mybir
from concourse.alu_op_type import AluOpType as Op
from concourse.bass_utils import run_bass_kernel_spmd

F32 = mybir.dt.float32
F16 = mybir.dt.float16
I32 = mybir.dt.int32
U32 = mybir.dt.uint32
U16 = mybir.dt.uint16
AF = mybir.ActivationFunctionType

H_IMG, W_IMG = 2048, 2048
N_CORES = 8
OUT_ROWS = H_IMG // N_CORES          # 256
T_ITERS = 8                           # fixed masked-dilate iterations
R_IMG = 276                           # local img rows
Y0, Y1 = 2, 274                       # img rows with weak/strong (272 rows)
RW = Y1 - Y0                          # 272
HH = RW + 2                           # 274: hysteresis rows (1 zero pad each side)
BASE_OFF = 10                         # local img row of first output row
NCHUNK = W_IMG // 128                 # 16 column chunks
NSTRIP = W_IMG // 16                  # 128 strips of 16 cols (+8 halo each side)
T1 = float(np.sqrt(2.0) - 1.0)        # tan(22.5 deg)
W_PAD = W_IMG + 2                     # 2050 (1 replicated col each side)
M23 = float(2 ** 23)
CS = [128, NCHUNK, R_IMG]             # col-layout shape


# ---------------------------------------------------------------- host consts
def _make_consts():
    c = {}
    # Vertical band matrices: out[n] = sum_k B[k, n] * in[global_row(k)]
    # B121: [1,2,1] smoothing; B101: out[n] = in[n+1] - in[n-1]
    b121 = np.zeros((128, 3, R_IMG), np.float16)
    b101 = np.zeros((128, 3, R_IMG), np.float16)
    for rc in range(3):
        for k in range(128):
            gr = 128 * rc + k
            if gr >= R_IMG:
                continue
            for n in range(1, R_IMG - 1):
                d = gr - n
                if d == -1 or d == 1:
                    b121[k, rc, n] = 1.0
                elif d == 0:
                    b121[k, rc, n] = 2.0
                if d == 1:
                    b101[k, rc, n] = 1.0
                elif d == -1:
                    b101[k, rc, n] = -1.0
    c["b121"] = b121
    c["b101"] = b101

    # Bit-pack matrices: strip s covers cols 16s-8 .. 16s+23 (bit b = col 16s-8+b)
    wlo = np.zeros((128, NCHUNK, 128), np.float16)
    whi = np.zeros((128, NCHUNK, 128), np.float16)
    for j in range(NCHUNK):
        for k in range(128):
            col = 128 * j + k
            for s in range(NSTRIP):
                b = col - 16 * s + 8
                if 0 <= b < 16:
                    wlo[k, j, s] = float(2 ** b)
                elif 16 <= b < 32:
                    whi[k, j, s] = float(2 ** (b - 16))
    c["wlo"] = wlo
    c["whi"] = whi
    return c


_CONSTS = None


def _consts():
    global _CONSTS
    if _CONSTS is None:
        _CONSTS = _make_consts()
    return _CONSTS


def _host_shards(x):
    """Per-core input shards + per-row uint32 penalty mask [128, HH]."""
    x = np.asarray(x, dtype=np.float32)
    shards = []
    for c in range(N_CORES):
        base = OUT_ROWS * c - BASE_OFF
        rows = np.clip(np.arange(base, base + R_IMG), 0, H_IMG - 1)
        xs = np.pad(x[rows], ((0, 0), (1, 1)), mode="edge").astype(np.float32)
        # hysteresis row h (1..HH-2) <-> img-local row h+1, global base+h+1
        glob = base + np.arange(HH) + 1
        ok = (glob >= 1) & (glob <= H_IMG - 2)
        ok[0] = False
        ok[HH - 1] = False
        pen = np.where(ok, np.uint32(0xFFFFFFFF), np.uint32(0))
        penrep = np.broadcast_to(pen[None, :], (128, HH)).copy()
        penrep[0, :] &= np.uint32(~(1 << 8) & 0xFFFFFFFF)     # col 0 border
        penrep[127, :] &= np.uint32(~(1 << 23) & 0xFFFFFFFF)  # col 2047 border
        shards.append((xs, penrep))
    return shards


# ---------------------------------------------------------------- device body
def _body(tc: tile.TileContext, io):
    nc = tc.nc
    x_d, pen_d, b121_d, b101_d, wlo_d, whi_d, out_d = io[:7]
    dbg = io[7] if len(io) > 7 else None
    R = R_IMG
    NQ = 4
    GQ = NCHUNK // NQ

    rc_rows = [(0, 128), (128, 128), (256, R - 256)]

    with ExitStack() as outer:
        singles = outer.enter_context(tc.tile_pool(name="consts", bufs=1))
        pfull = outer.enter_context(tc.tile_pool(name="pfull", bufs=1))
        pA = outer.enter_context(tc.tile_pool(name="pA", bufs=2))
        pAC = outer.enter_context(tc.tile_pool(name="pAC", bufs=3))
        pC = outer.enter_context(tc.tile_pool(name="pC", bufs=2))
        ph = outer.enter_context(tc.tile_pool(name="ph", bufs=1))
        pSh = outer.enter_context(tc.tile_pool(name="pSh", bufs=2))
        phorS = ExitStack()
        phor = phorS.enter_context(tc.tile_pool(name="phor", bufs=1))
        psumS = ExitStack()
        psum1 = psumS.enter_context(tc.tile_pool(name="psum1", bufs=2,
                                                 space="PSUM"))
        ppckS = ExitStack()
        ppck = ppckS.enter_context(tc.tile_pool(name="psumpk", bufs=1,
                                                space="PSUM"))

        # full-width persistent tiles
        mag = pfull.tile(CS, F16, tag="mag")

        # ------- phase 1: load, integerize (exact floor(255x)), horiz passes
        dT = phor.tile([128, 3, W_IMG], F16, tag="dT")
        sT = phor.tile([128, 3, W_IMG], F16, tag="sT")
        with ExitStack() as ph1:
            px = ph1.enter_context(tc.tile_pool(name="px", bufs=3))
            pw = ph1.enter_context(tc.tile_pool(name="pw", bufs=3))
            pimg = ph1.enter_context(tc.tile_pool(name="pimg", bufs=1))

            imgf = None
            if dbg is not None:
                imgf = pimg.tile([128, 3, W_PAD], F16, name="imgf",
                                 tag="img")
            for rc in (2, 0, 1):
                r0, nr = rc_rows[rc]
                img = None
                if dbg is None:
                    img = pimg.tile([128, W_PAD], F16, name="imgc",
                                    tag="imgc", bufs=2)
                if dbg is not None:
                    def imgv(a, b, rc=rc):
                        return imgf[:a, rc, b]
                else:
                    def imgv(a, b, img=img):
                        return img[:a, b]
                # column halves: integerize [0:1026] then [1026:2050]
                for cs, ce in ((0, 1026), (1026, W_PAD)):
                    ln = ce - cs
                    xt = px.tile([128, 1026], F32, tag="x")
                    nc.sync.dma_start(xt[:nr, :ln], x_d[r0:r0 + nr, cs:ce])
                    # y = fl(255*x) exactly as the reference computes it
                    yt = pw.tile([128, 1026], F32, tag="y")
                    nc.scalar.activation(yt[:nr, :ln], xt[:nr, :ln], AF.Copy,
                                         scale=255.0)
                    # exact floor(y): n = rne(y), img = n - (n > y)
                    n16 = pw.tile([128, 1026], F16, tag="n16")
                    nc.vector.tensor_scalar(n16[:nr, :ln], yt[:nr, :ln], M23,
                                            M23, Op.add, Op.subtract)
                    d16 = pw.tile([128, 1026], U16, tag="d16")
                    nc.vector.tensor_tensor(d16[:nr, :ln], n16[:nr, :ln],
                                            yt[:nr, :ln], Op.is_gt)
                    nc.vector.tensor_tensor(imgv(nr, slice(cs, ce)),
                                            n16[:nr, :ln], d16[:nr, :ln],
                                            Op.subtract)
                    # horizontal passes for the covered window
                    ws, we = (0, 1024) if cs == 0 else (1024, W_IMG)
                    wl = we - ws
                    nc.vector.tensor_tensor(dT[:nr, rc, ws:we],
                                            imgv(nr, slice(ws + 2, we + 2)),
                                            imgv(nr, slice(ws, we)),
                                            Op.subtract)
                    c2 = pw.tile([128, 1024], F16, tag="c2")
                    nc.scalar.activation(c2[:nr, :wl],
                                         imgv(nr, slice(ws + 1, we + 1)),
                                         AF.Copy, scale=2.0)
                    s1t = pw.tile([128, 1024], F16, tag="s1")
                    nc.gpsimd.tensor_tensor(s1t[:nr, :wl],
                                            imgv(nr, slice(ws, we)),
                                            imgv(nr, slice(ws + 2, we + 2)),
                                            Op.add)
                    nc.vector.tensor_tensor(sT[:nr, rc, ws:we],
                                            s1t[:nr, :wl], c2[:nr, :wl],
                                            Op.add)

            # ---- constants to SBUF (queued behind x loads on purpose)
            b121 = singles.tile([128, 3, R], F16)
            nc.sync.dma_start(b121[:], b121_d)
            b101 = singles.tile([128, 3, R], F16)
            nc.sync.dma_start(b101[:], b101_d)
            wlo = singles.tile([128, NCHUNK, 128], F16)
            nc.sync.dma_start(wlo[:], wlo_d)
            whi = singles.tile([128, NCHUNK, 128], F16)
            nc.sync.dma_start(whi[:], whi_d)
            pen = singles.tile([128, HH], U32)
            nc.sync.dma_start(pen[:], pen_d)
            sc1 = singles.tile([128, 1], U32)
            nc.vector.memset(sc1[:], 1)
            sc16 = singles.tile([128, 1], U32)
            nc.vector.memset(sc16[:], 16)
            zrow = singles.tile([1, R], F16)
            nc.vector.memset(zrow[:], 0.0)
            if dbg is not None:
                nc.sync.dma_start(dbg["img"], imgf[:])

        # pack accumulators (PSUM, accumulate across all 16 chunks)
        pk_wklo = ppck.tile([128, RW], F32, tag="wklo")
        pk_wkhi = ppck.tile([128, RW], F32, tag="wkhi")
        pk_stlo = ppck.tile([128, RW], F32, tag="stlo")
        pk_sthi = ppck.tile([128, RW], F32, tag="sthi")

        # ------- per-group NMS pipeline
        # /64-scaled domain: all magnitudes are k/64 (exact in f16, k<=2040)
        grp = [None] * NQ   # rotating per-group tiles

        def stage_a(q):
            j0 = GQ * q
            gxs = pA.tile([128, GQ, R], F16, tag="gxs")
            gys = pA.tile([128, GQ, R], F16, tag="gys")
            for jj in range(GQ):
                j = j0 + jj
                gxp = psum1.tile([128, R], F32, tag="gx")
                gyp = psum1.tile([128, R], F32, tag="gy")
                for rc, (r0, nr) in enumerate(rc_rows):
                    nc.tensor.matmul(gxp[:],
                                     dT[:nr, rc, 128 * j:128 * (j + 1)],
                                     b121[:nr, rc, :], start=(rc == 0),
                                     stop=(rc == 2))
                for rc, (r0, nr) in enumerate(rc_rows):
                    nc.tensor.matmul(gyp[:],
                                     sT[:nr, rc, 128 * j:128 * (j + 1)],
                                     b101[:nr, rc, :], start=(rc == 0),
                                     stop=(rc == 2))
                nc.scalar.activation(gxs[:, jj, :], gxp[:], AF.Copy,
                                     scale=1.0 / 64.0)
                nc.scalar.activation(gys[:, jj, :], gyp[:], AF.Copy,
                                     scale=1.0 / 64.0)
            absx = pA.tile([128, GQ, R], F16, tag="absx")
            nc.scalar.activation(absx[:], gxs[:], AF.Abs)
            absy = pA.tile([128, GQ, R], F16, tag="absy")
            nc.scalar.activation(absy[:], gys[:], AF.Abs)
            pm = pA.tile([128, GQ, R], F16, tag="pm")
            nc.gpsimd.tensor_tensor(pm[:], gxs[:], gys[:], Op.mult)
            sl = slice(j0, j0 + GQ)
            nc.vector.tensor_tensor(mag[:, sl, :], absx[:], absy[:], Op.add)

            d0 = pAC.tile([128, GQ, R], U16, tag="d0")
            nc.vector.scalar_tensor_tensor(d0[:], absx[:], T1, absy[:],
                                           Op.mult, Op.is_gt)
            d2 = pAC.tile([128, GQ, R], U16, tag="d2")
            nc.vector.scalar_tensor_tensor(d2[:], absy[:], T1, absx[:],
                                           Op.mult, Op.is_ge)
            pmneg = pAC.tile([128, GQ, R], U16, tag="pmneg")
            nc.vector.tensor_single_scalar(pmneg[:], pm[:], 0.0, Op.is_lt)
            grp[q] = {"d0": d0, "d2": d2, "pmneg": pmneg}

        def shifts(q):
            j0, j1 = GQ * q, GQ * (q + 1)
            for sa, tl, tr in ((mag, "magL", "magR"),):
                dl = pSh.tile([128, GQ, R], F16, name=tl + str(q), tag=tl)
                dr = pSh.tile([128, GQ, R], F16, name=tr + str(q), tag=tr)
                grp[q][tl] = dl
                grp[q][tr] = dr
                nc.sync.dma_start(dr[0:127, :, :], sa[1:128, j0:j1, :])
                nc.sync.dma_start(dr[127:128, 0:GQ - 1, :],
                                  sa[0:1, j0 + 1:j1, :])
                if q < NQ - 1:
                    nc.sync.dma_start(dr[127:128, GQ - 1:GQ, :],
                                      sa[0:1, j1:j1 + 1, :])
                else:
                    nc.sync.dma_start(dr[127:128, GQ - 1:GQ, :], zrow[:])
                nc.sync.dma_start(dl[1:128, :, :], sa[0:127, j0:j1, :])
                nc.sync.dma_start(dl[0:1, 1:GQ, :],
                                  sa[127:128, j0:j1 - 1, :])
                if q > 0:
                    nc.sync.dma_start(dl[0:1, 0:1, :],
                                      sa[127:128, j0 - 1:j0, :])
                else:
                    nc.vector.memset(dl[0:1, 0:1, :], 0.0)

        def G(t, q, dy=0):
            return t[:, GQ * q:GQ * (q + 1), Y0 + dy:Y1 + dy]

        def stage_c(q):
            g = grp[q]
            Mt = pC.tile([128, GQ, R], F16, tag="Mt")
            Mi3 = pC.tile([128, GQ, R], F16, tag="Mi3")
            M0 = pC.tile([128, GQ, R], F16, tag="M0")
            M2 = pC.tile([128, GQ, R], F16, tag="M2")

            def YG(t, dy=0):
                return t[:, :, Y0 + dy:Y1 + dy]

            mL, mR = g["magL"], g["magR"]
            qL = pC.tile([128, GQ, R], F16, tag="qL")
            nc.vector.tensor_scalar(qL[:], mL[:], 1.0 / 64.0, None, Op.add)
            qR = pC.tile([128, GQ, R], F16, tag="qR")
            nc.vector.tensor_scalar(qR[:], mR[:], 1.0 / 64.0, None, Op.add)
            qC = pC.tile([128, GQ, R], F16, tag="qC")
            nc.vector.tensor_scalar(qC[:], mag[:, GQ * q:GQ * (q + 1), :],
                                    1.0 / 64.0, None, Op.add)
            nc.vector.tensor_tensor(YG(Mt), YG(qL, 1), YG(mR, -1),
                                    Op.max)                              # 45
            nc.vector.tensor_tensor(YG(Mi3), YG(qR, 1), YG(mL, -1),
                                    Op.max)                              # 135
            nc.vector.tensor_tensor(YG(M0), YG(qL), YG(mR), Op.max)
            nc.vector.tensor_tensor(YG(M2), YG(qC, 1), G(mag, q, -1),
                                    Op.max)                              # 90
            nc.vector.copy_predicated(YG(Mt), YG(g["pmneg"]), YG(Mi3))
            nc.vector.copy_predicated(YG(Mt), YG(g["d0"]), YG(M0))
            nc.vector.copy_predicated(YG(Mt), YG(g["d2"]), YG(M2))
            Mw = pC.tile([128, GQ, R], F16, tag="Mw")
            nc.vector.tensor_scalar(YG(Mw), YG(Mt), 101.0 / 64.0, None,
                                    Op.max)
            weak = pC.tile([128, GQ, R], F16, tag="M0")
            nc.vector.tensor_tensor(YG(weak), YG(Mw), G(mag, q), Op.is_le)
            nc.vector.tensor_scalar(YG(Mi3), YG(Mw), 201.0 / 64.0, None,
                                    Op.max)
            strong = pC.tile([128, GQ, R], F16, tag="M2")
            nc.vector.tensor_tensor(YG(strong), YG(Mi3), G(mag, q), Op.is_le)

            j0 = GQ * q
            for jj in range(GQ):
                j = j0 + jj
                st_j = (j == 0)
                sp_j = (j == NCHUNK - 1)
                nc.tensor.matmul(pk_wklo[:], wlo[:, j, :],
                                 weak[:, jj, Y0:Y1], start=st_j, stop=sp_j,
                                 skip_group_check=True)
                nc.tensor.matmul(pk_wkhi[:], whi[:, j, :],
                                 weak[:, jj, Y0:Y1], start=st_j, stop=sp_j,
                                 skip_group_check=True)
                nc.tensor.matmul(pk_stlo[:], wlo[:, j, :],
                                 strong[:, jj, Y0:Y1], start=st_j, stop=sp_j,
                                 skip_group_check=True)
                nc.tensor.matmul(pk_sthi[:], whi[:, j, :],
                                 strong[:, jj, Y0:Y1], start=st_j, stop=sp_j,
                                 skip_group_check=True)

        # staggered emission: stageA(q+1) before shifts(q) (cross-group edges)
        stage_a(0)
        stage_a(1)
        shifts(0)
        stage_a(2)
        shifts(1)
        stage_c(0)
        stage_a(3)
        shifts(2)
        stage_c(1)
        shifts(3)
        stage_c(2)
        stage_c(3)
        phorS.close()

        # ------- combine packed halves into u32 words, apply border penalty
        wk32 = ph.tile([128, HH], U32, tag="wk")
        st32 = ph.tile([128, HH], U32, tag="st")
        lo32 = ph.tile([128, RW], U32, tag="lo32")
        hi32 = ph.tile([128, RW], U32, tag="hi32")
        nc.vector.tensor_copy(lo32[:], pk_wklo[:])
        nc.vector.tensor_copy(hi32[:], pk_wkhi[:])
        nc.vector.scalar_tensor_tensor(wk32[:, 1:HH - 1], hi32[:], sc16[:],
                                       lo32[:], Op.logical_shift_left,
                                       Op.bitwise_or)
        lo32b = ph.tile([128, RW], U32, tag="lo32b")
        hi32b = ph.tile([128, RW], U32, tag="hi32b")
        nc.vector.tensor_copy(lo32b[:], pk_stlo[:])
        nc.vector.tensor_copy(hi32b[:], pk_sthi[:])
        nc.vector.scalar_tensor_tensor(st32[:, 1:HH - 1], hi32b[:], sc16[:],
                                       lo32b[:], Op.logical_shift_left,
                                       Op.bitwise_or)
        ppckS.close()
        psumS.close()
        nc.vector.tensor_tensor(wk32[:, 1:HH - 1], wk32[:, 1:HH - 1],
                                pen[:, 1:HH - 1], Op.bitwise_and)
        nc.vector.tensor_tensor(st32[:, 1:HH - 1], st32[:, 1:HH - 1],
                                pen[:, 1:HH - 1], Op.bitwise_and)

        # ------- hysteresis: fixed masked-dilate iterations on packed words
        cur = st32
        nxt = ph.tile([128, HH], U32, tag="curB")
        at = ph.tile([128, HH], U32, tag="a")
        bt = ph.tile([128, HH], U32, tag="b")
        ut = ph.tile([128, HH], U32, tag="u")
        nc.vector.memset(bt[:, 0:1], 0)
        nc.vector.memset(bt[:, HH - 1:HH], 0)
        for it in range(T_ITERS):
            nc.vector.scalar_tensor_tensor(
                at[:, 1:HH - 1], cur[:, 1:HH - 1], sc1[:], cur[:, 1:HH - 1],
                Op.logical_shift_left, Op.bitwise_or)
            nc.vector.scalar_tensor_tensor(
                bt[:, 1:HH - 1], cur[:, 1:HH - 1], sc1[:], at[:, 1:HH - 1],
                Op.logical_shift_right, Op.bitwise_or)
            nc.vector.tensor_tensor(ut[:, 1:HH - 1], bt[:, 0:HH - 2],
                                    bt[:, 2:HH], Op.bitwise_or)
            nc.vector.tensor_tensor(ut[:, 1:HH - 1], ut[:, 1:HH - 1],
                                    bt[:, 1:HH - 1], Op.bitwise_or)
            nc.vector.tensor_tensor(nxt[:, 1:HH - 1], ut[:, 1:HH - 1],
                                    wk32[:, 1:HH - 1], Op.bitwise_and)
            cur, nxt = nxt, cur

        if dbg is not None:
            nc.sync.dma_start(dbg["wk32"], wk32[:])
            nc.sync.dma_start(dbg["st32"], st32[:])
            nc.sync.dma_start(dbg["mag"], mag[:])

        # ------- emit packed output rows (host unpacks bits)
        nc.sync.dma_start(out_d, cur[:, BASE_OFF - 1:BASE_OFF - 1 + OUT_ROWS])


def _build_nc(debug_out=False):
    nc = bacc.Bacc("TRN2", target_bir_lowering=False, debug=False,
                   num_devices=N_CORES)
    x_d = nc.dram_tensor("x", [R_IMG, W_PAD], F32, kind="ExternalInput").ap()
    pen_d = nc.dram_tensor("pen", [128, HH], U32, kind="ExternalInput").ap()
    b121_d = nc.dram_tensor("b121", [128, 3, R_IMG], F16, kind="ExternalInput").ap()
    b101_d = nc.dram_tensor("b101", [128, 3, R_IMG], F16, kind="ExternalInput").ap()
    wlo_d = nc.dram_tensor("wlo", [128, NCHUNK, 128], F16, kind="ExternalInput").ap()
    whi_d = nc.dram_tensor("whi", [128, NCHUNK, 128], F16, kind="ExternalInput").ap()
    out_d = nc.dram_tensor("out", [128, OUT_ROWS], U32, kind="ExternalOutput").ap()
    io = [x_d, pen_d, b121_d, b101_d, wlo_d, whi_d, out_d]
    if debug_out:
        dbg = {}
        for nm, shp, dt in [("wk32", [128, HH], U32), ("st32", [128, HH], U32),
                            ("mag", CS, F16),
                            ("img", [128, 3, W_PAD], F16)]:
            dbg[nm] = nc.dram_tensor("dbg_" + nm, shp, dt,
                                     kind="ExternalOutput").ap()
        io.append(dbg)
    with tile.TileContext(nc) as tc:
        _body(tc, io)
    nc.compile()
    return nc


_NC = None


def _get_nc():
    global _NC
    if _NC is None:
        _NC = _build_nc()
    return _NC


def _in_maps(x):
    cs = _consts()
    shards = _host_shards(x)
    maps = []
    for c in range(N_CORES):
        xs, pen = shards[c]
        maps.append({
            "x": xs, "pen": pen,
            "b121": cs["b121"], "b101": cs["b101"],
            "wlo": cs["wlo"], "whi": cs["whi"],
        })
    return maps


def _unpack_words(words):
    """[128, 256] u32 strip words -> [256, 2048] f32 edge map."""
    w = np.ascontiguousarray(words.astype('<u4'))
    by = w.view(np.uint8).reshape(128, OUT_ROWS, 4)[:, :, 1:3]
    bits = np.unpackbits(by, axis=2, bitorder="little")  # [128, 256, 16]
    return np.transpose(bits, (1, 0, 2)).reshape(OUT_ROWS, W_IMG)


LAST_RESULT = None


def kernel(x):
    global LAST_RESULT
    nc = _get_nc()
    maps = _in_maps(x)
    res = run_bass_kernel_spmd(nc, maps, list(range(N_CORES)))
    LAST_RESULT = res
    blocks = [_unpack_words(res.results[c]["out"]) for c in range(N_CORES)]
    edges = np.concatenate(blocks, axis=0).astype(np.float32)
    return np.broadcast_to(edges[None], (3, H_IMG, W_IMG))
